# revision 20
# baseline (speedup 1.0000x reference)
"""FJSP decoder kernel for Trainium2, data-parallel over batch on 8 NeuronCores.

Factorized attention (see derivation in the module docstring of the
reference): for s=(j,m), q/k/v atoms decompose as x[s] = xj[j] + xm[m], so
the joint softmax splits into an E-side (contraction over j', K=100) and an
F-side (contraction over m', K=20), and the multi-head combine collapses
through w2 = Wmhc @ Wshc into per-head scalars uv = v @ w2:

  [SE|Nj] = eA @ [eC^T | eC^T*uvj];  [SF|Nm] = eB @ [eD^T | eD^T*uvm]
  score1  = sum_h Nj/SE + Nm/SF (+bias);  p = softmax(10*tanh(score1/sqrt(D)))

Host folds Wv/Wmhc/Wshc/b_* into wvfold [2D,8] + one bias scalar, transposes
ej/em, and pre-pads the q-side weights into the 2-head-per-32-partition
window layout (head 2t at lanes 32t+0:16 in qE, head 2t+1 at 32t+16:32 in
qO, partner lanes zero; k stays compact since only one matmul operand needs
zeroed pad lanes).  Everything ships as ONE bf16 DMA.  All PE matmuls run
bf16 (1 cyc/col).  Per-head exps are merged in pairs into [100,480] PSUM
tiles to amortize ACT fixed cost; E-side uv scales go to DVE, F-side to
GPSIMD.  The combine is two DVE divides + one reduce; the logit tail is
tanh -> (optional mask add) -> exp(+accum) -> ones-matmul total -> one
fused scalar divide.
"""

import math

import numpy as np
import ml_dtypes

import concourse.bass as bass
import concourse.mybir as mybir
import concourse.tile as tile
from concourse.bass_utils import run_bass_kernel_spmd

F32 = mybir.dt.float32
BF16 = mybir.dt.bfloat16
AF = mybir.ActivationFunctionType
OP = mybir.AluOpType
AX = mybir.AxisListType

D, H, QD = 128, 8, 16
B, J, M = 8, 100, 20
INV_SQ = 1.0 / math.sqrt(QD)  # 0.25
SD = math.sqrt(D)

# data column layout (bf16): padded q weights, compact k weights, transposed
# activations, folded v weights, bias
QEJ, QEM, QOJ, QOM, KJ, KM = 0, 128, 256, 384, 512, 640
EJ, EM, MK, WVJ, WVM, BI = 768, 868, 888, 908, 916, 924
NCOL = 925

# pairs of heads sharing one PSUM tile; even-head pairs first so they are
# gated only by the qE copy
PAIRS = [(0, 2), (4, 6), (1, 3), (5, 7)]

# ---------------------------------------------------------------------------
# gen3 walrus accepts one sync-wait per instruction. Tile's kernel-tail
# drain accumulates one wait per active logical processor on a single
# Drain: spread them across engines (parallel waiting). Tile's semaphore
# pass can also attach >1 wait to ordinary instructions: shed extras onto
# same-engine NoOps inserted right before the offender.
_PATCHED = False


def _install_drain_patch():
    global _PATCHED
    if _PATCHED:
        return
    from concourse.tile import ScopedClock, TileContext

    def _split_drain_and_barrier(self, tick_clock, wait_clock):
        drain_inst = self.nc.sync.drain()
        wait_clock.add_sem_waits(
            drain_inst.ins, ScopedClock({None: tick_clock.global_clock})
        )
        si = drain_inst.ins.sync_info
        waits = list(si.on_wait) if si is not None else []
        if len(waits) > 1:
            assert not si.on_update
            sems = {s.name: s for s in self.sems.allocated().values()}
            drain_inst.ins.sync_info = None
            drain_inst.wait_op(sems[waits[0].ant_name], waits[0].wait_value, "sem-ge")
            engines = [
                self.nc.scalar,
                self.nc.vector,
                self.nc.tensor,
                self.nc.gpsimd,
                self.nc.sync,
            ]
            for i, w in enumerate(waits[1:]):
                extra = engines[i % len(engines)].drain()
                extra.wait_op(sems[w.ant_name], w.wait_value, "sem-ge")
        self.nc.all_engine_barrier()
        assert self.sems is not None
        popped = self.nc._tile_sem_poison_stack.pop()
        assert popped is self._sem_poison
        self.nc.clear_and_free_semaphores(list(self.sems.allocated().values()))

    TileContext._drain_and_barrier = _split_drain_and_barrier
    _PATCHED = True


def _split_multi_waits(nc):
    import bass_rust

    ctr = 0
    for fn in nc.m.functions:
        for bb in fn.blocks:
            il = bb.instructions
            if not any(
                i.sync_info is not None and len(i.sync_info.on_wait) > 1 for i in il
            ):
                continue
            new = []
            for ins in il:
                si = ins.sync_info
                if si is not None and len(si.on_wait) > 1:
                    waits = list(si.on_wait)
                    ups = list(si.on_update)
                    for w in waits[:-1]:
                        nop = mybir.InstNoOp(name=f"I-waitsplit-{ctr}", ins=[], outs=[])
                        ctr += 1
                        nop.engine = ins.engine
                        nop.sync_info = bass_rust.SyncInfo(on_update=[], on_wait=[w])
                        new.append(nop)
                    ins.sync_info = bass_rust.SyncInfo(
                        on_update=ups, on_wait=[waits[-1]]
                    )
                new.append(ins)
            bb.instructions = new


def _build(with_mask: bool):
    nc = bass.Bass()
    data_d = nc.dram_tensor("data", [D, NCOL], BF16, kind="ExternalInput")
    out_d = nc.dram_tensor("out", [J, M], F32, kind="ExternalOutput")

    with tile.TileContext(nc) as tc:
        with (
            tc.tile_pool(name="persist", bufs=1) as pp,
            tc.tile_pool(name="rot", bufs=4) as rp,
            tc.tile_pool(name="ps_misc", bufs=2, space="PSUM") as ps_misc,
            tc.tile_pool(name="ps_pair", bufs=4, space="PSUM") as ps_pair,
            tc.tile_pool(name="ps_fs", bufs=2, space="PSUM") as ps_fs,
        ):
            ones_sb = pp.tile([D, J], F32, tag="ones")
            nc.gpsimd.memset(ones_sb, 1.0)

            data_sb = pp.tile([D, NCOL], BF16, tag="data")
            nc.sync.dma_start(out=data_sb, in_=data_d[:])

            ejT = data_sb[:, EJ : EJ + J]
            emT = data_sb[:, EM : EM + M]
            mask_v = data_sb[0:J, MK : MK + M]
            wvj_v = data_sb[:, WVJ : WVJ + H]
            wvm_v = data_sb[:, WVM : WVM + H]
            bias_v = data_sb[0:J, BI : BI + 1]

            # ---- projections ------------------------------------------
            # two PSUM tiles: Tile serializes ALL readers of one PSUM tile,
            # so the k tile (read by ACT) is separate from the q/uv tile
            # (read by DVE).  Block layout: 0:100 job atoms, 100:120
            # machine atoms.
            pj_a = ps_misc.tile([D, 120], F32, tag="misc")
            nc.tensor.matmul(
                out=pj_a[:, 0:100], lhsT=data_sb[:, KJ : KJ + D], rhs=ejT
            )
            nc.tensor.matmul(
                out=pj_a[:, 100:120], lhsT=data_sb[:, KM : KM + D], rhs=emT
            )
            pj_b = ps_misc.tile([D, 256], F32, tag="misc")
            nc.tensor.matmul(
                out=pj_b[:, 0:100], lhsT=data_sb[:, QEJ : QEJ + D], rhs=ejT
            )
            nc.tensor.matmul(
                out=pj_b[:, 100:120], lhsT=data_sb[:, QEM : QEM + D], rhs=emT
            )
            nc.tensor.matmul(
                out=pj_b[:, 120:220], lhsT=data_sb[:, QOJ : QOJ + D], rhs=ejT
            )
            nc.tensor.matmul(
                out=pj_b[:, 220:240], lhsT=data_sb[:, QOM : QOM + D], rhs=emT
            )
            # uv[j,h] = (ej @ Wvfold_j), uv[m,8+h] = (em @ Wvfold_m)
            uv_ps = pj_b[:, 240:256]
            nc.tensor.matmul(out=uv_ps[0:J, 0:8], lhsT=ejT, rhs=wvj_v)
            nc.tensor.matmul(out=uv_ps[0:M, 8:16], lhsT=emT, rhs=wvm_v)

            ktt = pp.tile([D, 120], BF16, tag="ktt")
            nc.scalar.copy(out=ktt, in_=pj_a[:, :])  # kt on ACT
            qet = pp.tile([D, 120], BF16, tag="qet")
            nc.vector.tensor_copy(out=qet, in_=pj_b[:, 0:120])  # qE
            qot = pp.tile([D, 120], BF16, tag="qot")
            nc.vector.tensor_copy(out=qot, in_=pj_b[:, 120:240])  # qO
            kt = ktt[:, :]
            qE = qet[:, :]
            qO = qot[:, :]

            uv_sb = pp.tile([D, 16], F32, tag="uv")
            nc.vector.tensor_copy(out=uv_sb[0:J, :], in_=pj_b[0:J, 240:256])

            # ---- head loop: pair mms -> exp -> uv scales -> mm3/mm4 ----
            # per-head layout in e1 (stride 280):
            #   0:100 eA^T | 100:120 eC^T | 120:140 eC^T*uvj |
            #   140:240 eB^T | 240:260 eD^T | 260:280 eD^T*uvm
            pair_ps = []
            for ha, hb in PAIRS:
                ps = ps_pair.tile([D, 480], F32, tag="pair")
                for ci, h in enumerate((ha, hb)):
                    t = h // 2
                    qv = qE if h % 2 == 0 else qO
                    c0 = 240 * ci
                    nc.tensor.matmul(
                        out=ps[0:J, c0 : c0 + 120],
                        lhsT=kt[32 * t : 32 * t + 32, 0:100],
                        rhs=qv[32 * t : 32 * t + 32, 0:120],
                        tile_position=(32 * t, 0),
                    )
                    nc.tensor.matmul(
                        out=ps[0:M, c0 + 120 : c0 + 240],
                        lhsT=kt[32 * t : 32 * t + 32, 100:120],
                        rhs=qv[32 * t : 32 * t + 32, 0:120],
                        tile_position=(32 * t, 0),
                    )
                pair_ps.append(ps)

            # Each pair gets its OWN psum tile for [SF|Nm]/[SE|Nj] so the
            # per-pair reciprocal+multiply only depends on that pair's
            # matmuls (Tile tracks PSUM deps per tile, not per region) and
            # pipelines fully under the next pairs' exps.  fs layout:
            # [J, head-in-pair, (SF 0:20 | Nm 20:40 | SE 40:60 | Nj 60:80)]
            rall = pp.tile([D, 16, M], F32, tag="rall")
            dall = pp.tile([D, M, 16], F32, tag="dall")
            fs_tiles = []
            e1s = []

            def den_view(t3):  # [J,2,(g,20-of-40)] denominators SF,SE
                return t3.rearrange("p s (g x) -> p s g x", x=40)[:, :, :, 0:M]

            def num_view(t3):  # numerators Nm,Nj
                return t3.rearrange("p s (g x) -> p s g x", x=40)[:, :, :, M : 2 * M]

            def combine_pair(q):
                fs_q = fs_tiles[q][0:J, :, :]
                r_v = rall[0:J, 4 * q : 4 * q + 4, :].rearrange(
                    "p (s g) x -> p s g x", s=2
                )
                nc.vector.reciprocal(out=r_v, in_=den_view(fs_q))
                d_v = dall[0:J, :, 4 * q : 4 * q + 4].rearrange(
                    "p m (s g) -> p s g m", s=2
                )
                nc.vector.tensor_tensor(
                    out=d_v, in0=num_view(fs_q), in1=r_v, op=OP.mult
                )

            for p, (ha, hb) in enumerate(PAIRS):
                ps = pair_ps[p]
                e1 = rp.tile([D, 560], BF16, tag="e1")
                in_v = ps[0:J, 0:480].rearrange("p (a b x) -> p a b x", a=2, x=120)
                out_v = e1[0:J, 0:560].rearrange("p (a b y) -> p a b y", a=2, y=140)[
                    :, :, :, 0:120
                ]
                nc.scalar.activation(out=out_v, in_=in_v, func=AF.Exp, scale=INV_SQ)
                e1s.append(e1)
                with tc.high_priority():
                    for ci, h in enumerate((ha, hb)):
                        c0 = 280 * ci
                        nc.vector.tensor_scalar_mul(
                            out=e1[0:J, c0 + 120 : c0 + 140],
                            in0=e1[0:J, c0 + 100 : c0 + 120],
                            scalar1=uv_sb[0:J, h : h + 1],
                        )
                        nc.gpsimd.tensor_scalar_mul(
                            out=e1[0:M, c0 + 260 : c0 + 280],
                            in0=e1[0:M, c0 + 240 : c0 + 260],
                            scalar1=uv_sb[0:M, 8 + h : 9 + h],
                        )
                fs = ps_fs.tile([D, 2, 80], F32, tag="fs")
                fs_tiles.append(fs)
                for ci in range(2):
                    c0 = 280 * ci
                    nc.tensor.matmul(
                        out=fs[0:J, ci, 0:40],
                        lhsT=e1[0:M, c0 + 140 : c0 + 240],
                        rhs=e1[0:M, c0 + 240 : c0 + 280],
                    )
                    nc.tensor.matmul(
                        out=fs[0:J, ci, 40:80],
                        lhsT=e1[0:J, c0 : c0 + 100],
                        rhs=e1[0:J, c0 + 100 : c0 + 140],
                    )
                if p >= 1:
                    combine_pair(p - 1)
            combine_pair(3)

            c1 = pp.tile([D, M], F32, tag="c1")
            nc.vector.reduce_sum(out=c1[0:J, :], in_=dall[0:J, :, :], axis=AX.X)

            # ---- logits tail -------------------------------------------
            t_sb = pp.tile([D, M], F32, tag="t")
            nc.scalar.activation(
                out=t_sb[0:J, :], in_=c1[0:J, :], func=AF.Tanh,
                scale=1.0 / SD, bias=bias_v,
            )
            e_sb = pp.tile([J, M], F32, tag="e")
            s_row = pp.tile([J, 1], F32, tag="srow")
            if with_mask:
                arg = pp.tile([J, M], F32, tag="arg")
                nc.vector.scalar_tensor_tensor(
                    out=arg, in0=t_sb[0:J, :], scalar=10.0, in1=mask_v,
                    op0=OP.mult, op1=OP.add,
                )
                nc.scalar.activation(
                    out=e_sb, in_=arg, func=AF.Exp, scale=1.0, accum_out=s_row
                )
            else:
                nc.scalar.activation(
                    out=e_sb, in_=t_sb[0:J, :], func=AF.Exp, scale=10.0,
                    accum_out=s_row,
                )
            tot_ps = ps_misc.tile([D, 8], F32, tag="misc")
            nc.tensor.matmul(
                out=tot_ps[0:J, 0:1], lhsT=ones_sb[0:J, 0:J], rhs=s_row
            )
            rtot = pp.tile([J, 1], F32, tag="rtot")
            nc.vector.reciprocal(out=rtot, in_=tot_ps[0:J, 0:1])
            out_t = pp.tile([J, M], F32, tag="outt")
            nc.vector.tensor_scalar_mul(out=out_t, in0=e_sb, scalar1=rtot)
            nc.sync.dma_start(out=out_d[:], in_=out_t)

    _split_multi_waits(nc)
    return nc


_NC = None
_NC_MASKED = None
last_results = None


def _pack_weights(inputs):
    Wq3 = np.asarray(inputs["Wq3"], np.float32)
    Wk = np.asarray(inputs["Wk"], np.float32)
    Wv = np.asarray(inputs["Wv"], np.float32)
    Wmhc = np.asarray(inputs["Wmhc"], np.float32)
    Wshc = np.asarray(inputs["Wshc"], np.float32).reshape(D)
    b_mhc = np.asarray(inputs["b_mhc"], np.float32).reshape(D)
    b_shc = float(np.asarray(inputs["b_shc"]).reshape(-1)[0])

    w2 = Wmhc @ Wshc  # [128]
    bias_c = float(b_mhc @ Wshc + b_shc)
    wvf = (Wv * w2[None, :]).reshape(2 * D, H, QD).sum(-1)  # [256, 8]

    base = np.zeros((D, NCOL), np.float32)
    for t in range(4):
        he, ho = 2 * t, 2 * t + 1
        sl_e = slice(QD * he, QD * he + QD)
        sl_o = slice(QD * ho, QD * ho + QD)
        base[:, QEJ + 32 * t : QEJ + 32 * t + 16] = Wq3[:D, sl_e]
        base[:, QEM + 32 * t : QEM + 32 * t + 16] = Wq3[D:, sl_e]
        base[:, QOJ + 32 * t + 16 : QOJ + 32 * t + 32] = Wq3[:D, sl_o]
        base[:, QOM + 32 * t + 16 : QOM + 32 * t + 32] = Wq3[D:, sl_o]
    base[:, KJ : KJ + D] = Wk[:D]
    base[:, KM : KM + D] = Wk[D:]
    base[:, WVJ : WVJ + H] = wvf[:D]
    base[:, WVM : WVM + H] = wvf[D:]
    base[:, BI] = bias_c / SD
    return base


def kernel(**inputs):
    global _NC, _NC_MASKED, last_results
    _install_drain_patch()

    msks = np.asarray(inputs["ninf_mask"], np.float32)
    with_mask = bool(np.any(msks != 0.0))
    if with_mask:
        if _NC_MASKED is None:
            _NC_MASKED = _build(True)
        nc = _NC_MASKED
    else:
        if _NC is None:
            _NC = _build(False)
        nc = _NC

    base = _pack_weights(inputs)
    ejs = np.asarray(inputs["encoded_job"], np.float32)
    ems = np.asarray(inputs["encoded_machine"], np.float32)

    in_maps = []
    for b in range(B):
        d = base.copy()
        d[:, EJ : EJ + J] = ejs[b].T
        d[:, EM : EM + M] = ems[b].T
        d[0:J, MK : MK + M] = msks[b]
        in_maps.append({"data": d.astype(ml_dtypes.bfloat16)})

    last_results = run_bass_kernel_spmd(nc, in_maps, core_ids=list(range(B)))
    out = np.stack(
        [np.asarray(last_results.results[b]["out"]).reshape(J * M) for b in range(B)]
    )
    return out.astype(np.float32)
